# revision 4
# baseline (speedup 1.0000x reference)
"""CascadeRCNN head (3-stage cascade + test-time ensemble) on 8 Trainium2 NeuronCores.

v2: fp8(e3m4) datapath to beat the Tensor/DMA/gather bottlenecks of v1:
 - feats table, fc1/fc2 weights, pooled x, h1 stored fp8 (host pre-scales by
   powers of 2; exact to undo); matmuls run f8xf8 at bf16 PE rate.
 - fc1/fc2 weights shipped K-packed ([128, KT*N] layout) so each weight DMA
   moves 8-14KB per partition instead of 1-4KB rows -> no packet-rate limit.
 - ROIAlign gathers batched: ONE indirect DMA per bin-row (14 indices per
   partition) instead of 14 -> 21 gather instructions instead of 294.
 - fc1 matmuls interleaved with per-bin-row pooling so PE/DMA/DVE overlap.

Data-parallel over rois: 1000 rois sharded 8 x 125; feats and weights
replicated. Host only shards/concats.
"""

import numpy as np
from contextlib import ExitStack

import concourse.bass as bass
import concourse.tile as tile
from concourse import bacc, mybir
from concourse.masks import make_identity

F32 = mybir.dt.float32
BF16 = mybir.dt.bfloat16
F8 = mybir.dt.float8e4   # e4m3: required for DoubleRow matmuls
I32 = mybir.dt.int32
Alu = mybir.AluOpType
Act = mybir.ActivationFunctionType
DR = mybir.MatmulPerfMode.DoubleRow

N_CORES = 8
R = 125              # rois per core
POOL = 7
C = 256
K1 = 12544           # 7*7*256
KT1 = K1 // 128      # 98 k-tiles
HID = 1024
KT2 = HID // 128     # 8
NCLS = 81
IMG = 1024.0

# fp8 scales (powers of two; exact to undo)
S_F = 2.0            # feats/pooled-x scale
S_W1 = 128.0         # fc1 weight scale
S_H1 = 2.0           # h1 scale
S_W2 = 64.0          # fc2 weight scale
# psum1 = (S_F x)@(S_W1 W1) = 256 (x@W1); psum2 = (S_H1 h1)@(S_W2 W2) = 128 (h1@W2)
PS1 = S_F * S_W1
PS2 = S_H1 * S_W2

FEAT_ROWS = 256 * 256 + 128 * 128 + 64 * 64 + 32 * 32  # 87040
N_STAGES = 3


def _roi_prep(nc, pools, rois_t, grid_t):
    """From rois [R,4] compute gather indices and bilinear weights.

    Returns (idx_i32 [R, 49], wx_b [R,7] bf16, wy_eff [R,7]).
    idx free layout: (jy, jx) -> col jy*7 + jx. Each index is a row of the
    PAIRED feats table ([pixel, 512] = ch of (y,x) ++ ch of (y+1,x)); a 1KB
    f8 gather covers pixel columns (bx, bx+1) and both y rows: all 4 corners.
    """
    prep = pools["prep"]
    v = nc.vector

    def pt(cols, dtype=F32, tag=None):
        return prep.tile([R, cols], dtype, tag=tag, name=tag)

    y1 = rois_t[:, 0:1]
    x1 = rois_t[:, 1:2]
    y2 = rois_t[:, 2:3]
    x2 = rois_t[:, 3:4]

    hh = pt(1, tag="hh"); v.tensor_tensor(hh[:], y2, y1, op=Alu.subtract)
    ww = pt(1, tag="ww"); v.tensor_tensor(ww[:], x2, x1, op=Alu.subtract)
    hw = pt(1, tag="hw"); v.tensor_tensor(hw[:], hh[:], ww[:], op=Alu.mult)
    v.tensor_scalar(hw[:], hw[:], 1e-6, None, op0=Alu.max)

    # level selection: lvl = 2 + (hw>=112^2) + (hw>=224^2) + (hw>=448^2)
    g2 = pt(1, tag="g2"); v.tensor_scalar(g2[:], hw[:], 12544.0, None, op0=Alu.is_ge)
    g3 = pt(1, tag="g3"); v.tensor_scalar(g3[:], hw[:], 50176.0, None, op0=Alu.is_ge)
    g4 = pt(1, tag="g4"); v.tensor_scalar(g4[:], hw[:], 200704.0, None, op0=Alu.is_ge)

    # inv_stride = 0.25 - 0.125*g2 - 0.0625*g3 - 0.03125*g4  (exact)
    invs = pt(1, tag="invs")
    v.tensor_scalar(invs[:], g2[:], -0.125, 0.25, op0=Alu.mult, op1=Alu.add)
    t0 = pt(1, tag="t0")
    v.tensor_scalar(t0[:], g3[:], -0.0625, None, op0=Alu.mult)
    v.tensor_tensor(invs[:], invs[:], t0[:], op=Alu.add)
    v.tensor_scalar(t0[:], g4[:], -0.03125, None, op0=Alu.mult)
    v.tensor_tensor(invs[:], invs[:], t0[:], op=Alu.add)

    # feature side S = 1024 * inv_stride in {256,128,64,32}; level base offset
    S = pt(1, tag="S"); v.tensor_scalar(S[:], invs[:], 1024.0, None, op0=Alu.mult)
    base = pt(1, tag="base")
    v.tensor_scalar(base[:], g2[:], 65536.0, None, op0=Alu.mult)
    v.tensor_scalar(t0[:], g3[:], 16384.0, None, op0=Alu.mult)
    v.tensor_tensor(base[:], base[:], t0[:], op=Alu.add)
    v.tensor_scalar(t0[:], g4[:], 4096.0, None, op0=Alu.mult)
    v.tensor_tensor(base[:], base[:], t0[:], op=Alu.add)
    Sm1 = pt(1, tag="Sm1"); v.tensor_scalar(Sm1[:], S[:], -1.0, None, op0=Alu.add)
    Sm2 = pt(1, tag="Sm2"); v.tensor_scalar(Sm2[:], S[:], -2.0, None, op0=Alu.add)

    # scaled roi coords (exact: multiply by power of two)
    sy1 = pt(1, tag="sy1"); v.tensor_tensor(sy1[:], y1, invs[:], op=Alu.mult)
    sx1 = pt(1, tag="sx1"); v.tensor_tensor(sx1[:], x1, invs[:], op=Alu.mult)
    sy2 = pt(1, tag="sy2"); v.tensor_tensor(sy2[:], y2, invs[:], op=Alu.mult)
    sx2 = pt(1, tag="sx2"); v.tensor_tensor(sx2[:], x2, invs[:], op=Alu.mult)
    dy = pt(1, tag="dy"); v.tensor_tensor(dy[:], sy2[:], sy1[:], op=Alu.subtract)
    dx = pt(1, tag="dx"); v.tensor_tensor(dx[:], sx2[:], sx1[:], op=Alu.subtract)

    def axis_prep(scoord, dcoord, suffix, edge_clamp):
        # ys = grid*d + s  (matches ref rounding: mult then add)
        ys = pt(POOL, tag="ys" + suffix)
        v.tensor_scalar(ys[:], grid_t[0:R, :], dcoord[:], scoord[:],
                        op0=Alu.mult, op1=Alu.add)
        # robust floor (works under truncation or round-to-nearest casts)
        yi = pt(POOL, I32, tag="yi" + suffix)
        v.tensor_copy(yi[:], ys[:])
        yf = pt(POOL, tag="yf" + suffix)
        v.tensor_copy(yf[:], yi[:])
        gt = pt(POOL, tag="gt" + suffix)
        v.tensor_tensor(gt[:], yf[:], ys[:], op=Alu.is_gt)
        y0f = pt(POOL, tag="y0f" + suffix)
        v.tensor_tensor(y0f[:], yf[:], gt[:], op=Alu.subtract)
        # clip to [0, S-1] (ys >= 0 so lower clip is a no-op)
        y0c = pt(POOL, tag="y0c" + suffix)
        v.tensor_scalar(y0c[:], y0f[:], Sm1[:], None, op0=Alu.min)
        # weight: clip(ys - y0c, 0, 1)
        wy = pt(POOL, tag="wy" + suffix)
        v.tensor_tensor(wy[:], ys[:], y0c[:], op=Alu.subtract)
        v.tensor_scalar(wy[:], wy[:], 0.0, 1.0, op0=Alu.max, op1=Alu.min)
        if not edge_clamp:
            # paired table already bakes in the +1 clamp along this axis
            return y0c, wy
        # x axis: base col clamped to S-2; force w=1.0 at the right edge so
        # the second fetched pixel (the S-1 column) is selected
        by = pt(POOL, tag="by" + suffix)
        v.tensor_scalar(by[:], y0c[:], Sm2[:], None, op0=Alu.min)
        fl = pt(POOL, tag="fl" + suffix)
        v.tensor_scalar(fl[:], y0f[:], Sm1[:], None, op0=Alu.is_ge)
        v.tensor_tensor(wy[:], wy[:], fl[:], op=Alu.max)
        return by, wy

    by, wy_eff = axis_prep(sy1, dy, "Y", edge_clamp=False)
    bx, wx_eff = axis_prep(sx1, dx, "X", edge_clamp=True)

    # rowA = base + by*S  [R,7]
    rowA = pt(POOL, tag="rowA")
    v.tensor_scalar(rowA[:], by[:], S[:], base[:], op0=Alu.mult, op1=Alu.add)

    # idx_f layout [R, 7jy, 7jx]: pixel row index = base + by*S + bx
    idxf = pools["idx"].tile([R, POOL * POOL], F32, tag="idxf", name="idxf")
    v3 = idxf[:].rearrange("p (a b) -> p a b", a=POOL, b=POOL)
    v.tensor_tensor(v3,
                    rowA[:, :, None].to_broadcast([R, POOL, POOL]),
                    bx[:, None, :].to_broadcast([R, POOL, POOL]),
                    op=Alu.add)
    idx_i32 = pools["idx"].tile([R, POOL * POOL], I32, tag="idxi", name="idxi")
    v.tensor_copy(idx_i32[:], idxf[:])
    # wx as bf16 for the bf16 x-interp multiply
    wx_b = pools["prep"].tile([R, POOL], BF16, tag="wxb", name="wxb")
    v.tensor_copy(wx_b[:], wx_eff[:])
    return idx_i32, wx_b, wy_eff


def _pool_jy(nc, pools, feats_ap, idx_i32, wx_b, wy_eff, ident_b, xt, jy):
    """Gather + bilinear-interp + transpose for one bin-row jy.

    Fills xt (tile [128, 14*128], f8, K-block t at cols t*128, 125 used).
    Gathered 1KB layout per bin jx: [x0y0 ch, x0y1 ch, x1y0 ch, x1y1 ch].
    """
    v = nc.vector
    G = pools["gath"].tile([R, POOL * 1024], F8, tag="G", name="G")
    for j in range(POOL):
        nc.gpsimd.indirect_dma_start(
            out=G[:, j * 1024:(j + 1) * 1024], out_offset=None,
            in_=feats_ap[:],
            in_offset=bass.IndirectOffsetOnAxis(
                ap=idx_i32[:, jy * POOL + j:jy * POOL + j + 1], axis=0),
        )
    gv = G[:].rearrange("p (b x y e) -> p b x y e", b=POOL, x=2, y=2, e=C)
    # y-interp into bf16 T[b, x, ch]: T = G0 + wy*(G1 - G0)
    T = pools["interp"].tile([R, POOL * 512], BF16, tag="T", name="T")
    tv = T[:].rearrange("p (b x e) -> p b x e", b=POOL, x=2, e=C)
    v.tensor_tensor(tv, gv[:, :, :, 1, :], gv[:, :, :, 0, :], op=Alu.subtract)
    v.tensor_scalar(tv, tv, wy_eff[:, jy:jy + 1], None, op0=Alu.mult)
    G0b = pools["interp"].tile([R, POOL * 512], BF16, tag="G0b", name="G0b")
    g0v = G0b[:].rearrange("p (b x e) -> p b x e", b=POOL, x=2, e=C)
    nc.scalar.copy(g0v, gv[:, :, :, 0, :])
    v.tensor_tensor(tv, tv, g0v, op=Alu.add)
    # x-interp: P = T0 + wx*(T1 - T0)   (bf16)
    P = pools["interp"].tile([R, POOL * 256], BF16, tag="P", name="P")
    pv = P[:].rearrange("p (b e) -> p b e", b=POOL, e=C)
    v.tensor_tensor(tv[:, :, 1, :], tv[:, :, 1, :], tv[:, :, 0, :],
                    op=Alu.subtract)
    v.tensor_tensor(tv[:, :, 1, :], tv[:, :, 1, :],
                    wx_b[:, :, None].to_broadcast([R, POOL, C]),
                    op=Alu.mult)
    v.tensor_tensor(pv, tv[:, :, 0, :], tv[:, :, 1, :], op=Alu.add)
    _pool_jy.last_G = G
    # transpose the 14 K-blocks into xt (f8), 4-at-a-time PSUM drains
    for t0 in range(0, 14, 4):
        nt = min(4, 14 - t0)
        ps = pools["pt"].tile([128, 4 * 128], BF16, space="PSUM", tag="ptr",
                              name="ptr")
        for u in range(nt):
            t = t0 + u
            nc.tensor.transpose(out=ps[:, u * 128:u * 128 + R],
                                in_=P[:, t * 128:(t + 1) * 128],
                                identity=ident_b[0:R, 0:R])
        # full-width contiguous drain (pad cols 125..127 carry junk, never
        # read: matmul lhsT slices stop at col 125)
        nc.vector.tensor_copy(xt[:, t0 * 128:(t0 + nt) * 128],
                              ps[:, 0:nt * 128])


def _fc1_chunk(nc, pools, xt, jy, w1p_ap, head, psums, first):
    """fc1 matmuls of bin-row jy: 7 DoubleRow k-pair matmuls x 2 psum halves.

    DoubleRow packs 2 fp8 k-rows per PE cell: lhsT/rhs get a 3D AP whose
    middle axis (size 2) walks the k-tile pair; one matmul contracts K=256.
    """
    wg = pools["w1"].tile([128, 14 * HID], F8, tag="w1", name="w1")
    nc.sync.dma_start(wg[:], w1p_ap[head, :, jy * 14 * HID:(jy + 1) * 14 * HID])
    xt3 = xt[:].rearrange("p (t c) -> p t c", t=14, c=128)
    wg3 = wg[:].rearrange("p (t n) -> p t n", t=14, n=HID)
    for u in range(7):
        for j, psum in enumerate(psums):
            nc.tensor.matmul(
                psum[:],
                lhsT=xt3[:, 2 * u:2 * u + 2, 0:R],
                rhs=wg3[:, 2 * u:2 * u + 2, j * 512:(j + 1) * 512],
                perf_mode=DR,
                start=(first and u == 0), stop=False)


def _bias_finish(nc, pools, psums, bias_ap, head, ones_b, n_out, relu, scale,
                 out_dt, sizes):
    """Add bias (bf16 matmul) into psums, close group, scaled act -> h tile."""
    bt = pools["bias"].tile([1, n_out], BF16, tag=f"bias{n_out}",
                            name=f"bias{n_out}")
    nc.scalar.dma_start(bt[:], bias_ap[head][None, :])
    for j, (psum, sz) in enumerate(zip(psums, sizes)):
        nc.tensor.matmul(psums[j][:, 0:sz], lhsT=ones_b[0:1, 0:R],
                         rhs=bt[0:1, j * 512:j * 512 + sz],
                         start=False, stop=True)
    h = pools["h"].tile([R, n_out], out_dt, tag=f"h{n_out}", name=f"h{n_out}")
    for j, (psum, sz) in enumerate(zip(psums, sizes)):
        nc.scalar.activation(h[:, j * 512:j * 512 + sz], psum[:, 0:sz],
                             Act.Relu if relu else Act.Copy, scale=scale)
    return h


def _transpose_h(nc, pools, h, ident_b, out_dt):
    """h [R, 1024] bf16 -> hT [128, 1024] out_dt (8 blocks, 125 cols each)."""
    tg = "hT8" if out_dt == F8 else "hTb"
    hT = pools["ht"].tile([128, HID], out_dt, tag=tg, name=tg)
    for t0 in range(0, KT2, 4):
        ps = pools["pt"].tile([128, 4 * 128], BF16, space="PSUM", tag="ptr",
                              name="ptr")
        for u in range(4):
            t = t0 + u
            nc.tensor.transpose(out=ps[:, u * 128:u * 128 + R],
                                in_=h[:, t * 128:(t + 1) * 128],
                                identity=ident_b[0:R, 0:R])
        nc.vector.tensor_copy(hT[:, t0 * 128:(t0 + 4) * 128], ps[:, 0:512])
    return hT


def _fc_small(nc, pools, hT, w_ap, head, n_out, kt, wdt, pool_tag):
    """out_psums = hT.T @ W (packed [128, kt*n_out] layout), no bias yet."""
    wt = pools[pool_tag].tile([128, kt * n_out], wdt, tag=f"{pool_tag}{n_out}",
                              name=f"{pool_tag}{n_out}")
    nc.scalar.dma_start(wt[:], w_ap[head, :, :])
    offs = list(range(0, n_out, 512))
    sizes = [min(512, n_out - o) for o in offs]
    psums = [pools["pfc"].tile([R, 512], F32, space="PSUM", tag=f"ps{j}",
                               name=f"ps{j}") for j in range(len(offs))]
    if wdt == F8 and kt % 2 == 0:
        hT3 = hT[:].rearrange("p (t c) -> p t c", t=kt, c=128)
        wt3 = wt[:].rearrange("p (t n) -> p t n", t=kt, n=n_out)
        for u in range(kt // 2):
            for j, (o, sz) in enumerate(zip(offs, sizes)):
                nc.tensor.matmul(
                    psums[j][:, 0:sz],
                    lhsT=hT3[:, 2 * u:2 * u + 2, 0:R],
                    rhs=wt3[:, 2 * u:2 * u + 2, o:o + sz],
                    perf_mode=DR,
                    start=(u == 0), stop=False)
        return psums, sizes
    for t in range(kt):
        for j, (o, sz) in enumerate(zip(offs, sizes)):
            nc.tensor.matmul(
                psums[j][:, 0:sz],
                lhsT=hT[:, t * 128:t * 128 + R],
                rhs=wt[:, t * n_out + o:t * n_out + o + sz],
                start=(t == 0), stop=False)
    return psums, sizes


def _softmax(nc, pools, logits):
    v = nc.vector
    rmax = pools["prep"].tile([R, 1], F32, tag="rmax", name="rmax")
    v.tensor_reduce(rmax[:], logits[:], axis=mybir.AxisListType.X, op=Alu.max)
    nmax = pools["prep"].tile([R, 1], F32, tag="nmax", name="nmax")
    v.tensor_scalar(nmax[:], rmax[:], -1.0, None, op0=Alu.mult)
    e = pools["h"].tile([R, NCLS], F32, tag="smx", name="smx")
    nc.scalar.activation(e[:], logits[:], Act.Exp, bias=nmax[:], scale=1.0)
    ssum = pools["prep"].tile([R, 1], F32, tag="ssum", name="ssum")
    v.tensor_reduce(ssum[:], e[:], axis=mybir.AxisListType.X, op=Alu.add)
    rsum = pools["prep"].tile([R, 1], F32, tag="rsum", name="rsum")
    v.reciprocal(rsum[:], ssum[:])
    v.tensor_scalar(e[:], e[:], rsum[:], None, op0=Alu.mult)
    return e


def _delta2bbox(nc, pools, rois_t, deltas, stds_t, rois_pool):
    """rois_next = delta2bbox(rois_t, deltas) following the reference op order."""
    v = nc.vector
    prep = pools["prep"]

    def pt(tag):
        return prep.tile([R, 1], F32, tag=tag, name=tag)

    d = prep.tile([R, 4], F32, tag="dsc", name="dsc")
    v.tensor_tensor(d[:], deltas[:], stds_t[0:R, :], op=Alu.mult)
    y1 = rois_t[:, 0:1]; x1 = rois_t[:, 1:2]; y2 = rois_t[:, 2:3]; x2 = rois_t[:, 3:4]
    hh = pt("b_h"); v.tensor_tensor(hh[:], y2, y1, op=Alu.subtract)
    ww = pt("b_w"); v.tensor_tensor(ww[:], x2, x1, op=Alu.subtract)
    hh2 = pt("b_h2"); v.tensor_scalar(hh2[:], hh[:], 0.5, None, op0=Alu.mult)
    cy = pt("b_cy"); v.tensor_tensor(cy[:], y1, hh2[:], op=Alu.add)
    t = pt("b_t"); v.tensor_tensor(t[:], d[:, 0:1], hh[:], op=Alu.mult)
    v.tensor_tensor(cy[:], cy[:], t[:], op=Alu.add)
    ww2 = pt("b_w2"); v.tensor_scalar(ww2[:], ww[:], 0.5, None, op0=Alu.mult)
    cx = pt("b_cx"); v.tensor_tensor(cx[:], x1, ww2[:], op=Alu.add)
    v.tensor_tensor(t[:], d[:, 1:2], ww[:], op=Alu.mult)
    v.tensor_tensor(cx[:], cx[:], t[:], op=Alu.add)
    eh = pt("b_eh"); nc.scalar.activation(eh[:], d[:, 2:3], Act.Exp)
    ew = pt("b_ew"); nc.scalar.activation(ew[:], d[:, 3:4], Act.Exp)
    v.tensor_tensor(hh[:], hh[:], eh[:], op=Alu.mult)
    v.tensor_tensor(ww[:], ww[:], ew[:], op=Alu.mult)
    v.tensor_scalar(hh2[:], hh[:], 0.5, None, op0=Alu.mult)
    v.tensor_scalar(ww2[:], ww[:], 0.5, None, op0=Alu.mult)
    rn = rois_pool.tile([R, 4], F32, tag="rois", name="rois")
    v.tensor_tensor(rn[:, 0:1], cy[:], hh2[:], op=Alu.subtract)
    v.tensor_tensor(rn[:, 1:2], cx[:], ww2[:], op=Alu.subtract)
    v.tensor_tensor(rn[:, 2:3], cy[:], hh2[:], op=Alu.add)
    v.tensor_tensor(rn[:, 3:4], cx[:], ww2[:], op=Alu.add)
    for j in range(4):
        v.tensor_scalar(rn[:, j:j + 1], rn[:, j:j + 1], 0.0, IMG,
                        op0=Alu.max, op1=Alu.min)
    return rn


def build_kernel(ctx: ExitStack, tc: "tile.TileContext", aps: dict):
    nc = tc.nc
    pools = {}
    for name, bufs, space in [
        ("const", 1, "SBUF"), ("rois", 2, "SBUF"), ("prep", 2, "SBUF"),
        ("idx", 2, "SBUF"), ("gath", 2, "SBUF"), ("interp", 2, "SBUF"),
        ("xt", 1, "SBUF"), ("w1", 3, "SBUF"), ("w2", 2, "SBUF"),
        ("wsm", 2, "SBUF"), ("bias", 2, "SBUF"), ("h", 2, "SBUF"),
        ("ht", 2, "SBUF"), ("acc", 1, "SBUF"),
        ("pt", 2, "PSUM"), ("pfc", 2, "PSUM"), ("pkw", 1, "PSUM"),
    ]:
        pools[name] = ctx.enter_context(tc.tile_pool(name=name, bufs=bufs,
                                                     space=space))

    ident = pools["const"].tile([128, 128], F32, tag="ident", name="ident")
    make_identity(nc, ident[:])
    ident_b = pools["const"].tile([128, 128], BF16, tag="identb", name="identb")
    nc.vector.tensor_copy(ident_b[:], ident[:])
    ones_b = pools["const"].tile([1, 128], BF16, tag="onesb", name="onesb")
    nc.vector.memset(ones_b[:], 1.0)
    ones_f = pools["const"].tile([1, 128], F32, tag="onesf", name="onesf")
    nc.vector.memset(ones_f[:], 1.0)
    ones8 = pools["const"].tile([1, 128], F8, tag="ones8", name="ones8")
    nc.vector.memset(ones8[:], 1.0)
    grid_t = pools["const"].tile([128, POOL], F32, tag="grid", name="grid")
    nc.sync.dma_start(grid_t[:], aps["grid_c"][:])
    stds_t = pools["const"].tile([128, 4], F32, tag="stds", name="stds")
    nc.sync.dma_start(stds_t[:], aps["stds_c"][:])

    rois_t = pools["rois"].tile([R, 4], F32, tag="rois", name="rois")
    nc.sync.dma_start(rois_t[:], aps["rois"][:])

    def head_tail(head, want):
        """fc2 + cls/reg (+softmax) given psums of fc1 already accumulated."""
        # h1 = S_H1 * relu(x@W1 + b1): psum1 holds PS1*(x@W1); bias shipped
        # pre-scaled by PS1; act scale = S_H1/PS1
        h1 = _bias_finish(nc, pools, head_tail.psums, aps["fc1_bs"], head,
                          ones_b, HID, True, S_H1 / PS1, BF16, [512, 512])
        h1T = _transpose_h(nc, pools, h1, ident_b, F8)
        psums2, _ = _fc_small(nc, pools, h1T, aps["w2p"], head, HID, KT2, F8,
                              "w2")
        h2 = _bias_finish(nc, pools, psums2, aps["fc2_bs"], head, ones_b,
                          HID, True, 1.0 / PS2, BF16, [512, 512])
        h2T = _transpose_h(nc, pools, h2, ident_b, BF16)
        if want == "deltas":
            psd, _ = _fc_small(nc, pools, h2T, aps["regp"], head, 4, KT2,
                               BF16, "wsm")
            return _bias_finish(nc, pools, psd, aps["reg_b"], head, ones_b,
                                4, False, 1.0, F32, [4])
        psc, _ = _fc_small(nc, pools, h2T, aps["clsp"], head, NCLS, KT2,
                           BF16, "wsm")
        logits = _bias_finish(nc, pools, psc, aps["cls_b"], head, ones_b,
                              NCLS, False, 1.0, F32, [NCLS])
        return _softmax(nc, pools, logits)

    def fc1_psums():
        return [pools["pfc"].tile([R, 512], F32, space="PSUM", tag=f"ps{j}",
                                  name=f"ps{j}") for j in range(2)]

    acc = pools["acc"].tile([R, NCLS], F32, tag="acc", name="acc")
    xt_tiles = None
    for s in range(N_STAGES):
        # keep-warm chain: tiny matmuls dep-chained through the pooling
        # phase so the PE HAM never sees a fully idle window (transposes
        # don't count as PE-busy for HAM)
        kw = pools["pkw"].tile([R, 4], F32, space="PSUM", tag="kw", name="kw")
        nc.tensor.matmul(kw[:], lhsT=ones_f[0:1, 0:R], rhs=rois_t[0:1, 0:4],
                         start=True, stop=False)
        idx_i32, wx_b, wy_eff = _roi_prep(nc, pools, rois_t, grid_t)
        nc.tensor.matmul(kw[:], lhsT=ones_b[0:1, 0:R], rhs=wx_b[0:1, 0:4],
                         start=False, stop=False)
        xt_tiles = [pools["xt"].tile([128, 14 * 128], F8, tag=f"xt{j}",
                                     name=f"xt{j}") for j in range(POOL)]
        head_tail.psums = fc1_psums()
        for jy in range(POOL):
            _pool_jy(nc, pools, aps["feats8"], idx_i32, wx_b, wy_eff,
                     ident_b, xt_tiles[jy], jy)
            nc.tensor.matmul(kw[:], lhsT=ones8[0:1, 0:R],
                             rhs=_pool_jy.last_G[0:1, 0:4],
                             start=False, stop=(jy == POOL - 1))
            _fc1_chunk(nc, pools, xt_tiles[jy], jy, aps["w1p"], s,
                       head_tail.psums, first=(jy == 0))
        if s < 2:
            deltas = head_tail(s, "deltas")
            rois_t = _delta2bbox(nc, pools, rois_t, deltas, stds_t,
                                 pools["rois"])
        else:
            p3 = head_tail(2, "probs")
            nc.vector.tensor_copy(acc[:], p3[:])

    for i in range(2):
        head_tail.psums = fc1_psums()
        for jy in range(POOL):
            _fc1_chunk(nc, pools, xt_tiles[jy], jy, aps["w1p"], i,
                       head_tail.psums, first=(jy == 0))
        pi = head_tail(i, "probs")
        nc.vector.tensor_tensor(acc[:], acc[:], pi[:], op=Alu.add)

    outp = pools["h"].tile([R, NCLS], F32, tag="outp", name="outp")
    nc.vector.tensor_scalar(outp[:], acc[:], 1.0 / 3.0, None, op0=Alu.mult)
    nc.sync.dma_start(aps["out"][:], outp[:])


# ---------------------------------------------------------------------------
# host side
# ---------------------------------------------------------------------------

_CACHE: dict = {}


def build_program(reps: int = 1):
    nc = bacc.Bacc("TRN2", target_bir_lowering=False, debug=False,
                   num_devices=N_CORES)
    aps = {
        "feats8": nc.dram_tensor("feats8", [FEAT_ROWS, 2 * C], F8,
                                 kind="ExternalInput").ap(),
        "rois": nc.dram_tensor("rois", [R, 4], F32, kind="ExternalInput").ap(),
        "w1p": nc.dram_tensor("w1p", [3, 128, KT1 * HID], F8,
                              kind="ExternalInput").ap(),
        "w2p": nc.dram_tensor("w2p", [3, 128, KT2 * HID], F8,
                              kind="ExternalInput").ap(),
        "clsp": nc.dram_tensor("clsp", [3, 128, KT2 * NCLS], BF16,
                               kind="ExternalInput").ap(),
        "regp": nc.dram_tensor("regp", [3, 128, KT2 * 4], BF16,
                               kind="ExternalInput").ap(),
        "fc1_bs": nc.dram_tensor("fc1_bs", [3, HID], BF16,
                                 kind="ExternalInput").ap(),
        "fc2_bs": nc.dram_tensor("fc2_bs", [3, HID], BF16,
                                 kind="ExternalInput").ap(),
        "cls_b": nc.dram_tensor("cls_b", [3, NCLS], BF16,
                                kind="ExternalInput").ap(),
        "reg_b": nc.dram_tensor("reg_b", [3, 4], BF16,
                                kind="ExternalInput").ap(),
        "grid_c": nc.dram_tensor("grid_c", [128, POOL], F32,
                                 kind="ExternalInput").ap(),
        "stds_c": nc.dram_tensor("stds_c", [128, 4], F32,
                                 kind="ExternalInput").ap(),
        "out": nc.dram_tensor("out", [R, NCLS], F32,
                              kind="ExternalOutput").ap(),
    }
    with tile.TileContext(nc) as tc:
        for _ in range(reps):
            with ExitStack() as ctx:
                build_kernel(ctx, tc, aps)
    nc.compile()
    return nc


def host_consts():
    grid = ((np.arange(POOL, dtype=np.float32) + np.float32(0.5))
            / np.float32(POOL))
    grid_c = np.broadcast_to(grid, (128, POOL)).copy()
    stds_c = np.broadcast_to(
        np.array([0.1, 0.1, 0.2, 0.2], dtype=np.float32), (128, 4)).copy()
    return grid_c, stds_c


def _pack_k(w, kt, scale, dtype):
    """[3, kt*128, n] -> [3, 128, kt*n] (partition = k % 128 within tile)."""
    import ml_dtypes
    n = w.shape[-1]
    wp = np.asarray(w, np.float32).reshape(3, kt, 128, n)
    wp = np.ascontiguousarray(wp.transpose(0, 2, 1, 3)).reshape(3, 128, kt * n)
    return (wp * scale).astype(dtype)


def make_in_maps(inputs: dict) -> list:
    import ml_dtypes
    F8H = ml_dtypes.float8_e4m3
    BFH = ml_dtypes.bfloat16
    f32 = lambda x: np.ascontiguousarray(np.asarray(x, dtype=np.float32))

    def pair_level(p):
        # [S,S,C] -> [S*S, 2C]: row (y,x) = [feat(y,x), feat(min(y+1,S-1),x)]
        f = f32(p)[0]
        up = np.concatenate([f[1:], f[-1:]], axis=0)
        return np.concatenate([f, up], axis=-1).reshape(-1, 2 * C)

    feats_pair = np.concatenate(
        [pair_level(inputs[k]) for k in ("P2", "P3", "P4", "P5")], axis=0)
    grid_c, stds_c = host_consts()
    rois = f32(inputs["rois"])
    u8 = lambda x: x.view(np.uint8)
    u16 = lambda x: x.view(np.uint16)
    shared = {
        "feats8": u8((feats_pair * S_F).astype(F8H)),
        "w1p": u8(_pack_k(inputs["fc1_w"], KT1, S_W1, F8H)),
        "w2p": u8(_pack_k(inputs["fc2_w"], KT2, S_W2, F8H)),
        "clsp": u16(_pack_k(inputs["cls_w"], KT2, 1.0, BFH)),
        "regp": u16(_pack_k(inputs["reg_w"], KT2, 1.0, BFH)),
        "fc1_bs": u16((f32(inputs["fc1_b"]) * PS1).astype(BFH)),
        "fc2_bs": u16((f32(inputs["fc2_b"]) * PS2).astype(BFH)),
        "cls_b": u16(f32(inputs["cls_b"]).astype(BFH)),
        "reg_b": u16(f32(inputs["reg_b"]).astype(BFH)),
        "grid_c": grid_c, "stds_c": stds_c,
    }
    return [dict(shared, rois=rois[c * R:(c + 1) * R]) for c in range(N_CORES)]


def make_runner(nc):
    """Jitted SPMD executor: rois/outputs sharded over cores, all other
    inputs replicated (avoids the 8x host-side concat of the big weights)."""
    import jax
    from jax.sharding import Mesh, PartitionSpec
    from jax.experimental.shard_map import shard_map
    from concourse import bass2jax

    bass2jax.install_neuronx_cc_hook()
    pname = nc.partition_id_tensor.name if nc.partition_id_tensor else None
    in_names, out_names, out_avals = [], [], []
    for alloc in nc.m.functions[0].allocations:
        if not isinstance(alloc, mybir.MemoryLocationSet):
            continue
        name = alloc.memorylocations[0].name
        if alloc.kind == "ExternalInput":
            if name != pname:
                in_names.append(name)
        elif alloc.kind == "ExternalOutput":
            out_names.append(name)
            out_avals.append(jax.core.ShapedArray(
                tuple(alloc.tensor_shape), mybir.dt.np(alloc.dtype)))
    n_outs = len(out_avals)
    names_full = list(in_names) + out_names + ([pname] if pname else [])

    def _body(*args):
        ops = list(args)
        if pname is not None:
            ops.append(bass2jax.partition_id_tensor())
        return tuple(bass2jax._bass_exec_p.bind(
            *ops, out_avals=tuple(out_avals), in_names=tuple(names_full),
            out_names=tuple(out_names), lowering_input_output_aliases=(),
            sim_require_finite=True, sim_require_nnan=True, nc=nc))

    devices = jax.devices()[:N_CORES]
    mesh = Mesh(np.asarray(devices), ("core",))
    P_ = PartitionSpec
    in_specs = tuple(P_("core") if nm == "rois" else P_() for nm in in_names) \
        + (P_("core"),) * n_outs
    sharded = jax.jit(
        shard_map(_body, mesh=mesh, in_specs=in_specs,
                  out_specs=(P_("core"),) * n_outs, check_rep=False),
        keep_unused=True)

    def _args(shared, rois_full):
        args = [rois_full if nm == "rois" else shared[nm] for nm in in_names]
        args += [np.zeros((N_CORES * a.shape[0], *a.shape[1:]), a.dtype)
                 for a in out_avals]
        return args

    def prepare(shared: dict, rois_full: np.ndarray):
        from jax.sharding import NamedSharding
        args = _args(shared, rois_full)
        shards = [NamedSharding(mesh, s) for s in in_specs]
        return [jax.device_put(a, s) for a, s in zip(args, shards)]

    def run_dev(dev_args):
        out = sharded(*dev_args)
        jax.block_until_ready(out)
        return np.asarray(out[0])

    def run(shared: dict, rois_full: np.ndarray):
        out = sharded(*_args(shared, rois_full))
        jax.block_until_ready(out)
        return np.asarray(out[0])

    run.prepare = prepare
    run.run_dev = run_dev
    return run


def kernel(**inputs) -> np.ndarray:
    if "nc" not in _CACHE:
        _CACHE["nc"] = build_program()
        _CACHE["run"] = make_runner(_CACHE["nc"])
    in_maps = make_in_maps(inputs)
    shared = dict(in_maps[0])
    rois_full = np.ascontiguousarray(np.asarray(inputs["rois"], np.float32))
    out = _CACHE["run"](shared, rois_full)
    return out.astype(np.float32)
